# revision 22
# baseline (speedup 1.0000x reference)
"""CropAndResize (TF semantics, bilinear) Trainium2 Bass kernel.

Contract: kernel(image, boxes, box_ind) -> [512, 256, 14, 14] float32.

Strategy (box-sharded, balanced, over 8 NeuronCores):
  - Host: boxes sorted by box_ind and dealt out in equal contiguous
    groups of 64 per core (each group touches at most K adjacent
    batches; K=2 for the benchmark inputs).  Host pre-builds, per
    batch, a row-pair fp16 scratch table in DRAM laid out
    [y, x, ypar, c]: slot (y*100+x) holds rows y and y+1 of column x
    (bottom half of row 99 zeroed).  One 2KB gather element at slot
    (y0*100+x0) covers all four bilinear corners.  Host also computes
    per-pixel gather slot indices (int16, slot-sorted per core) and
    the four corner weights.
  - Device: per chunk of up to 1024 pixels, one dma_gather from the
    DRAM scratch, then a fused 4-term combine on DVE
    (tensor_scalar_mul + 3x scalar_tensor_tensor), fp16 result rows
    written straight back to DRAM (4KB per partition per chunk).
  - Host: inverse-permute the per-core fp16 rows into the full
    [512, 256, 14, 14] f32 output.
"""

import os
import numpy as np

H = 100
W = 100
C = 256
CROP = 14
P = 128
NCORES = 8
CHQ = 8  # q-columns (128 pixels) per chunk; 1024 idx/gather fits single-packet
SLOTS_B = H * W + 4  # per-batch slot count incl. pad slots
SLOT = 2 * C  # fp16 elems per (y,x) slot: [ypar, c]

_PROGRAM_CACHE = {}
LAST_RESULT = None  # BassKernelResults stash for test harness introspection


def _build_program(NCH, tail_q, K):
    import concourse.bacc as bacc
    import concourse.bass as bass
    import concourse.mybir as mybir
    import concourse.tile as tile
    from concourse._compat import get_trn_type

    NQ = (NCH - 1) * CHQ + tail_q
    NSLOT = K * SLOTS_B
    f32 = mybir.dt.float32
    f16 = mybir.dt.float16
    i16 = mybir.dt.int16

    # Two SWDGE queues + a 2x descriptor ring so gather N+1's Q7 descriptor
    # generation overlaps gather N's DMA drain (one 1024-idx gather fills a
    # default 16KB ring exactly, serializing gen behind transfer).
    nc = bacc.Bacc(
        get_trn_type() or "TRN2",
        target_bir_lowering=False,
        debug=False,
        num_swdge_queues=3,
        dynamic_dma_scratch_size=49152,
    )

    scr = nc.dram_tensor("scr", [NSLOT * SLOT], f16, kind="ExternalInput")
    tidx = nc.dram_tensor("tidx", [P, NCH * 64], i16, kind="ExternalInput")
    wts = nc.dram_tensor("wts", [P, 4 * NQ], f32, kind="ExternalInput")
    out = nc.dram_tensor("out", [NCH, P, CHQ, C], f16, kind="ExternalOutput")

    src_ap = bass.AP(scr[:].tensor, 0, [[SLOT, NSLOT - 1], [1, 2 * SLOT]])

    with tile.TileContext(nc) as tc:
        with (
            tc.tile_pool(name="static", bufs=1) as pool_s,
            tc.tile_pool(name="gath", bufs=6) as pool_g,
            tc.tile_pool(name="valp", bufs=4) as pool_v,
            tc.tile_pool(name="tmpp", bufs=2) as pool_t,
        ):
            # warmup gather: pays the ~6us Q7 gather-ucode IRAM load up front,
            # overlapped with the wts/tidx loads and ACT table load.
            ti0 = pool_s.tile([P, 1], i16)
            nc.vector.memset(ti0[:], 0)
            G0 = pool_s.tile([P, 1, 2 * SLOT], f16)
            nc.gpsimd.dma_gather(
                G0[:], src_ap, ti0[:], 16, 16, 2 * SLOT, elem_step=SLOT, queue_num=1
            )
            wts_sb = pool_s.tile([P, 4 * NQ], f32)
            nc.sync.dma_start(wts_sb[:], wts[:])
            tidx_sb = pool_s.tile([P, NCH * 64], i16)
            nc.sync.dma_start(tidx_sb[:], tidx[:])

            for ch in range(NCH):
                nq = CHQ if ch < NCH - 1 else tail_q
                ni = nq * P
                G = pool_g.tile([P, nq, 2 * SLOT], f16, tag="G")
                nc.gpsimd.dma_gather(
                    G[:],
                    src_ap,
                    tidx_sb[:, ch * 64 : ch * 64 + ni // 16],
                    ni,
                    ni,
                    2 * SLOT,
                    elem_step=SLOT,
                    queue_num=ch % 3,
                )
                # G per pixel: [x0:(top,bot), x1:(top,bot)] x 256c
                # -> tl=G[...,0:256] bl=[256:512] tr=[512:768] br=[768:1024]
                # tensor_scalar_mul streams one packed-f16 tensor (fast DVE
                # mode); the accumulate runs as three whole-chunk adds.
                val = pool_v.tile([P, nq, C], f16, tag="val")
                PA = pool_t.tile([P, nq, C], f16, tag="PA")
                PB = pool_t.tile([P, nq, C], f16, tag="PB")
                PC = pool_t.tile([P, nq, C], f16, tag="PC")
                PD = pool_t.tile([P, nq, C], f16, tag="PD")
                for q in range(nq):
                    col = ch * CHQ + q
                    w00 = wts_sb[:, 0 * NQ + col : 0 * NQ + col + 1]
                    w10 = wts_sb[:, 1 * NQ + col : 1 * NQ + col + 1]
                    w01 = wts_sb[:, 2 * NQ + col : 2 * NQ + col + 1]
                    w11 = wts_sb[:, 3 * NQ + col : 3 * NQ + col + 1]
                    nc.vector.tensor_scalar_mul(PA[:, q, :], G[:, q, 0:C], w00)
                    # DVE ops run ~45% slower than ACT's while Q7 desc-gen
                    # hogs the shared SBUF port; shift 2 of 16 muls to ACT.
                    if q in (3, 7):
                        nc.scalar.mul(PB[:, q, :], G[:, q, C : 2 * C], w10)
                    else:
                        nc.vector.tensor_scalar_mul(PB[:, q, :], G[:, q, C : 2 * C], w10)
                    nc.scalar.mul(PC[:, q, :], G[:, q, 2 * C : 3 * C], w01)
                    nc.scalar.mul(PD[:, q, :], G[:, q, 3 * C : 4 * C], w11)
                nc.vector.tensor_add(PA[:], PA[:], PB[:])
                nc.vector.tensor_add(PC[:], PC[:], PD[:])
                nc.vector.tensor_add(val[:], PA[:], PC[:])
                if nq == CHQ:
                    nc.sync.dma_start(out[ch], val[:])
                else:
                    nc.sync.dma_start(out[ch][:, 0:nq, :], val[:])

    nc.compile()
    return nc


def _host_prep(boxes):
    """Per-pixel gather slots + corner weights. Matches reference f32 math."""
    f32 = np.float32
    boxes = np.asarray(boxes, dtype=f32)
    y1, x1, y2, x2 = boxes[:, 0], boxes[:, 1], boxes[:, 2], boxes[:, 3]
    hs = np.arange(CROP, dtype=f32)
    ws = np.arange(CROP, dtype=f32)
    in_y = y1[:, None] * f32(H - 1) + hs[None, :] * (
        (y2 - y1) * f32(H - 1) / f32(CROP - 1)
    )[:, None]
    in_x = x1[:, None] * f32(W - 1) + ws[None, :] * (
        (x2 - x1) * f32(W - 1) / f32(CROP - 1)
    )[:, None]
    valid_y = (in_y >= 0) & (in_y <= H - 1)
    valid_x = (in_x >= 0) & (in_x <= W - 1)
    yc = np.clip(in_y, 0.0, H - 1)
    xc = np.clip(in_x, 0.0, W - 1)
    y0 = np.floor(yc)
    x0 = np.floor(xc)
    ly = (yc - y0).astype(f32)
    lx = (xc - x0).astype(f32)

    N = boxes.shape[0]
    slot = (y0.astype(np.int64)[:, :, None] * W + x0.astype(np.int64)[:, None, :]).astype(
        np.int32
    )
    one = f32(1.0)
    lyv = ly[:, :, None]
    lxv = lx[:, None, :]
    ones = np.ones((N, CROP, CROP), f32)
    w00 = (one - lxv) * (one - lyv) * ones  # tl
    w01 = lxv * (one - lyv) * ones  # tr
    w10 = (one - lxv) * lyv * ones  # bl
    w11 = lxv * lyv * ones  # br
    vmask = (valid_y[:, :, None] & valid_x[:, None, :]).astype(f32)
    for w in (w00, w01, w10, w11):
        w *= vmask
    return slot, w00, w10, w01, w11  # order: tl, bl, tr, br


def kernel(image, boxes, box_ind):
    global LAST_RESULT
    from concourse.bass_utils import run_bass_kernel_spmd

    image = np.asarray(image, dtype=np.float32)
    boxes = np.asarray(boxes, dtype=np.float32)
    box_ind = np.asarray(box_ind)
    NBATCH = image.shape[0]
    NBOX = boxes.shape[0]
    NPPB = CROP * CROP

    slot, w00, w10, w01, w11 = _host_prep(boxes)

    # balanced contiguous groups over box_ind-sorted order
    order = np.argsort(box_ind, kind="stable")
    per = (NBOX + NCORES - 1) // NCORES
    groups = [order[c * per : (c + 1) * per] for c in range(NCORES)]
    batch_lists = [np.unique(box_ind[g]).tolist() for g in groups]
    K = max(len(b) for b in batch_lists)
    NQ = (max(len(g) for g in groups) * NPPB + P - 1) // P
    NCH = (NQ + CHQ - 1) // CHQ
    tail_q = NQ - (NCH - 1) * CHQ
    NPIX = NQ * P

    # per-batch fp16 row-pair scratch [slot(y,x), ypar, c]
    timg = np.ascontiguousarray(
        image.transpose(0, 2, 3, 1).reshape(NBATCH, H * W, C)
    ).astype(np.float16)
    scr_all = np.zeros((NBATCH, SLOTS_B, SLOT), np.float16)
    scr_all[:, : H * W, :C] = timg
    scr_all[:, : (H - 1) * W, C:] = timg[:, W:]

    in_maps = []
    perms = []
    for c in range(NCORES):
        g = groups[c]
        nb = len(g)
        npx = nb * NPPB
        bs = batch_lists[c]
        bpad = bs + [bs[-1]] * (K - len(bs))
        lb = np.searchsorted(np.asarray(bs), box_ind[g])  # local batch idx
        gslot = lb[:, None].astype(np.int64) * SLOTS_B + slot[g].reshape(nb, NPPB)
        gslot = gslot.reshape(-1)
        assert gslot.max() < 32767 - 1, "int16 gather index overflow"
        perm = np.argsort(gslot, kind="stable")
        perms.append(perm)
        tflat = np.zeros(NPIX, np.int16)
        tflat[:npx] = gslot[perm]
        wflat = np.zeros((4, NPIX), np.float32)
        for r, w in enumerate((w00, w10, w01, w11)):
            wflat[r, :npx] = w[g].reshape(-1)[perm]

        tdev = np.zeros((P, NCH * 64), np.int16)
        for ch in range(NCH):
            nq = CHQ if ch < NCH - 1 else tail_q
            seg = tflat[ch * CHQ * P : ch * CHQ * P + nq * P]
            wrap = np.tile(seg.reshape(-1, 16).T, (8, 1))  # [128, nq*8]
            tdev[:, ch * 64 : ch * 64 + nq * 8] = wrap
        wdev = np.ascontiguousarray(
            wflat.reshape(4, NQ, P).transpose(2, 0, 1).reshape(P, 4 * NQ)
        )
        in_maps.append(
            {
                "scr": np.ascontiguousarray(scr_all[bpad]).reshape(-1),
                "tidx": tdev,
                "wts": wdev,
            }
        )

    key = (NCH, tail_q, K)
    if key not in _PROGRAM_CACHE:
        _PROGRAM_CACHE[key] = _build_program(NCH, tail_q, K)
    nc = _PROGRAM_CACHE[key]

    trace = bool(os.environ.get("BASS_TRACE"))
    res = run_bass_kernel_spmd(
        nc,
        in_maps,
        core_ids=list(range(NCORES)),
        trace=trace,
        trace_cores=list(range(NCORES)) if trace else None,
    )
    LAST_RESULT = res

    full = np.empty((NBOX, C, CROP, CROP), np.float32)
    for c in range(NCORES):
        g = groups[c]
        nb = len(g)
        if nb == 0:
            continue
        npx = nb * NPPB
        o = res.results[c]["out"]  # [NCH, P, CHQ, C] f16
        rows = o.transpose(0, 2, 1, 3).reshape(-1, C)[:npx].astype(np.float32)
        unsorted = np.empty_like(rows)
        unsorted[perms[c]] = rows
        r = unsorted.reshape(nb, CROP, CROP, C)
        full[g] = r.transpose(0, 3, 1, 2)
    return full


# revision 23
# speedup vs baseline: 1.0154x; 1.0154x over previous
"""CropAndResize (TF semantics, bilinear) Trainium2 Bass kernel.

Contract: kernel(image, boxes, box_ind) -> [512, 256, 14, 14] float32.

Strategy (box-sharded, balanced, over 8 NeuronCores):
  - Host: boxes sorted by box_ind and dealt out in equal contiguous
    groups of 64 per core (each group touches at most K adjacent
    batches; K=2 for the benchmark inputs).  Host pre-builds, per
    batch, a row-pair fp16 scratch table in DRAM laid out
    [y, x, ypar, c]: slot (y*100+x) holds rows y and y+1 of column x
    (bottom half of row 99 zeroed).  One 2KB gather element at slot
    (y0*100+x0) covers all four bilinear corners.  Host also computes
    per-pixel gather slot indices (int16, slot-sorted per core) and
    the four corner weights.
  - Device: per chunk of up to 1024 pixels, one dma_gather from the
    DRAM scratch, then a fused 4-term combine on DVE
    (tensor_scalar_mul + 3x scalar_tensor_tensor), fp16 result rows
    written straight back to DRAM (4KB per partition per chunk).
  - Host: inverse-permute the per-core fp16 rows into the full
    [512, 256, 14, 14] f32 output.
"""

import os
import numpy as np

H = 100
W = 100
C = 256
CROP = 14
P = 128
NCORES = 8
CHQ = 8  # q-columns (128 pixels) per chunk; 1024 idx/gather fits single-packet
SLOTS_B = H * W + 4  # per-batch slot count incl. pad slots
SLOT = 2 * C  # fp16 elems per (y,x) slot: [ypar, c]

_PROGRAM_CACHE = {}
LAST_RESULT = None  # BassKernelResults stash for test harness introspection


def _build_program(NCH, tail_q, K):
    import concourse.bacc as bacc
    import concourse.bass as bass
    import concourse.mybir as mybir
    import concourse.tile as tile
    from concourse._compat import get_trn_type

    NQ = (NCH - 1) * CHQ + tail_q
    NSLOT = K * SLOTS_B
    f32 = mybir.dt.float32
    f16 = mybir.dt.float16
    i16 = mybir.dt.int16

    # Two SWDGE queues + a 2x descriptor ring so gather N+1's Q7 descriptor
    # generation overlaps gather N's DMA drain (one 1024-idx gather fills a
    # default 16KB ring exactly, serializing gen behind transfer).
    nc = bacc.Bacc(
        get_trn_type() or "TRN2",
        target_bir_lowering=False,
        debug=False,
        num_swdge_queues=2,
        dynamic_dma_scratch_size=32768,
    )

    scr = nc.dram_tensor("scr", [NSLOT * SLOT], f16, kind="ExternalInput")
    tidx = nc.dram_tensor("tidx", [P, NCH * 64], i16, kind="ExternalInput")
    wts = nc.dram_tensor("wts", [P, 4 * NQ], f32, kind="ExternalInput")
    out = nc.dram_tensor("out", [NCH, P, CHQ, C], f16, kind="ExternalOutput")

    src_ap = bass.AP(scr[:].tensor, 0, [[SLOT, NSLOT - 1], [1, 2 * SLOT]])

    with tile.TileContext(nc) as tc:
        with (
            tc.tile_pool(name="static", bufs=1) as pool_s,
            tc.tile_pool(name="gath", bufs=5) as pool_g,
            tc.tile_pool(name="valp", bufs=4) as pool_v,
            tc.tile_pool(name="tmpp", bufs=3) as pool_t,
        ):
            # warmup gather: pays the ~6us Q7 gather-ucode IRAM load up front,
            # overlapped with the wts/tidx loads and ACT table load.
            ti0 = pool_s.tile([P, 1], i16)
            nc.vector.memset(ti0[:], 0)
            G0 = pool_s.tile([P, 1, 2 * SLOT], f16)
            nc.gpsimd.dma_gather(
                G0[:], src_ap, ti0[:], 16, 16, 2 * SLOT, elem_step=SLOT, queue_num=1
            )
            wts_sb = pool_s.tile([P, 4 * NQ], f32)
            nc.sync.dma_start(wts_sb[:], wts[:])
            tidx_sb = pool_s.tile([P, NCH * 64], i16)
            nc.sync.dma_start(tidx_sb[:], tidx[:])

            for ch in range(NCH):
                nq = CHQ if ch < NCH - 1 else tail_q
                ni = nq * P
                G = pool_g.tile([P, nq, 2 * SLOT], f16, tag="G")
                nc.gpsimd.dma_gather(
                    G[:],
                    src_ap,
                    tidx_sb[:, ch * 64 : ch * 64 + ni // 16],
                    ni,
                    ni,
                    2 * SLOT,
                    elem_step=SLOT,
                    queue_num=ch % 2,
                )
                # G per pixel: [x0:(top,bot), x1:(top,bot)] x 256c
                # -> tl=G[...,0:256] bl=[256:512] tr=[512:768] br=[768:1024]
                # tensor_scalar_mul streams one packed-f16 tensor (fast DVE
                # mode); the accumulate runs as three whole-chunk adds.
                val = pool_v.tile([P, nq, C], f16, tag="val")
                PA = pool_t.tile([P, nq, C], f16, tag="PA")
                PB = pool_t.tile([P, nq, C], f16, tag="PB")
                PC = pool_t.tile([P, nq, C], f16, tag="PC")
                PD = pool_t.tile([P, nq, C], f16, tag="PD")
                for q in range(nq):
                    col = ch * CHQ + q
                    w00 = wts_sb[:, 0 * NQ + col : 0 * NQ + col + 1]
                    w10 = wts_sb[:, 1 * NQ + col : 1 * NQ + col + 1]
                    w01 = wts_sb[:, 2 * NQ + col : 2 * NQ + col + 1]
                    w11 = wts_sb[:, 3 * NQ + col : 3 * NQ + col + 1]
                    nc.vector.tensor_scalar_mul(PA[:, q, :], G[:, q, 0:C], w00)
                    # DVE ops run ~45% slower than ACT's while Q7 desc-gen
                    # hogs the shared SBUF port; shift 2 of 16 muls to ACT.
                    if q in (3, 7):
                        nc.scalar.mul(PB[:, q, :], G[:, q, C : 2 * C], w10)
                    else:
                        nc.vector.tensor_scalar_mul(PB[:, q, :], G[:, q, C : 2 * C], w10)
                    nc.scalar.mul(PC[:, q, :], G[:, q, 2 * C : 3 * C], w01)
                    nc.scalar.mul(PD[:, q, :], G[:, q, 3 * C : 4 * C], w11)
                nc.vector.tensor_add(PA[:], PA[:], PB[:])
                nc.vector.tensor_add(PC[:], PC[:], PD[:])
                nc.vector.tensor_add(val[:], PA[:], PC[:])
                if nq == CHQ:
                    nc.sync.dma_start(out[ch], val[:])
                else:
                    nc.sync.dma_start(out[ch][:, 0:nq, :], val[:])

    nc.compile()
    return nc


def _host_prep(boxes):
    """Per-pixel gather slots + corner weights. Matches reference f32 math."""
    f32 = np.float32
    boxes = np.asarray(boxes, dtype=f32)
    y1, x1, y2, x2 = boxes[:, 0], boxes[:, 1], boxes[:, 2], boxes[:, 3]
    hs = np.arange(CROP, dtype=f32)
    ws = np.arange(CROP, dtype=f32)
    in_y = y1[:, None] * f32(H - 1) + hs[None, :] * (
        (y2 - y1) * f32(H - 1) / f32(CROP - 1)
    )[:, None]
    in_x = x1[:, None] * f32(W - 1) + ws[None, :] * (
        (x2 - x1) * f32(W - 1) / f32(CROP - 1)
    )[:, None]
    valid_y = (in_y >= 0) & (in_y <= H - 1)
    valid_x = (in_x >= 0) & (in_x <= W - 1)
    yc = np.clip(in_y, 0.0, H - 1)
    xc = np.clip(in_x, 0.0, W - 1)
    y0 = np.floor(yc)
    x0 = np.floor(xc)
    ly = (yc - y0).astype(f32)
    lx = (xc - x0).astype(f32)

    N = boxes.shape[0]
    slot = (y0.astype(np.int64)[:, :, None] * W + x0.astype(np.int64)[:, None, :]).astype(
        np.int32
    )
    one = f32(1.0)
    lyv = ly[:, :, None]
    lxv = lx[:, None, :]
    ones = np.ones((N, CROP, CROP), f32)
    w00 = (one - lxv) * (one - lyv) * ones  # tl
    w01 = lxv * (one - lyv) * ones  # tr
    w10 = (one - lxv) * lyv * ones  # bl
    w11 = lxv * lyv * ones  # br
    vmask = (valid_y[:, :, None] & valid_x[:, None, :]).astype(f32)
    for w in (w00, w01, w10, w11):
        w *= vmask
    return slot, w00, w10, w01, w11  # order: tl, bl, tr, br


def kernel(image, boxes, box_ind):
    global LAST_RESULT
    from concourse.bass_utils import run_bass_kernel_spmd

    image = np.asarray(image, dtype=np.float32)
    boxes = np.asarray(boxes, dtype=np.float32)
    box_ind = np.asarray(box_ind)
    NBATCH = image.shape[0]
    NBOX = boxes.shape[0]
    NPPB = CROP * CROP

    slot, w00, w10, w01, w11 = _host_prep(boxes)

    # balanced contiguous groups over box_ind-sorted order
    order = np.argsort(box_ind, kind="stable")
    per = (NBOX + NCORES - 1) // NCORES
    groups = [order[c * per : (c + 1) * per] for c in range(NCORES)]
    batch_lists = [np.unique(box_ind[g]).tolist() for g in groups]
    K = max(len(b) for b in batch_lists)
    NQ = (max(len(g) for g in groups) * NPPB + P - 1) // P
    NCH = (NQ + CHQ - 1) // CHQ
    tail_q = NQ - (NCH - 1) * CHQ
    NPIX = NQ * P

    # per-batch fp16 row-pair scratch [slot(y,x), ypar, c]
    timg = np.ascontiguousarray(
        image.transpose(0, 2, 3, 1).reshape(NBATCH, H * W, C)
    ).astype(np.float16)
    scr_all = np.zeros((NBATCH, SLOTS_B, SLOT), np.float16)
    scr_all[:, : H * W, :C] = timg
    scr_all[:, : (H - 1) * W, C:] = timg[:, W:]

    in_maps = []
    perms = []
    for c in range(NCORES):
        g = groups[c]
        nb = len(g)
        npx = nb * NPPB
        bs = batch_lists[c]
        bpad = bs + [bs[-1]] * (K - len(bs))
        lb = np.searchsorted(np.asarray(bs), box_ind[g])  # local batch idx
        gslot = lb[:, None].astype(np.int64) * SLOTS_B + slot[g].reshape(nb, NPPB)
        gslot = gslot.reshape(-1)
        assert gslot.max() < 32767 - 1, "int16 gather index overflow"
        perm = np.argsort(gslot, kind="stable")
        perms.append(perm)
        tflat = np.zeros(NPIX, np.int16)
        tflat[:npx] = gslot[perm]
        wflat = np.zeros((4, NPIX), np.float32)
        for r, w in enumerate((w00, w10, w01, w11)):
            wflat[r, :npx] = w[g].reshape(-1)[perm]

        tdev = np.zeros((P, NCH * 64), np.int16)
        for ch in range(NCH):
            nq = CHQ if ch < NCH - 1 else tail_q
            seg = tflat[ch * CHQ * P : ch * CHQ * P + nq * P]
            wrap = np.tile(seg.reshape(-1, 16).T, (8, 1))  # [128, nq*8]
            tdev[:, ch * 64 : ch * 64 + nq * 8] = wrap
        wdev = np.ascontiguousarray(
            wflat.reshape(4, NQ, P).transpose(2, 0, 1).reshape(P, 4 * NQ)
        )
        in_maps.append(
            {
                "scr": np.ascontiguousarray(scr_all[bpad]).reshape(-1),
                "tidx": tdev,
                "wts": wdev,
            }
        )

    key = (NCH, tail_q, K)
    if key not in _PROGRAM_CACHE:
        _PROGRAM_CACHE[key] = _build_program(NCH, tail_q, K)
    nc = _PROGRAM_CACHE[key]

    trace = bool(os.environ.get("BASS_TRACE"))
    res = run_bass_kernel_spmd(
        nc,
        in_maps,
        core_ids=list(range(NCORES)),
        trace=trace,
        trace_cores=list(range(NCORES)) if trace else None,
    )
    LAST_RESULT = res

    full = np.empty((NBOX, C, CROP, CROP), np.float32)
    for c in range(NCORES):
        g = groups[c]
        nb = len(g)
        if nb == 0:
            continue
        npx = nb * NPPB
        o = res.results[c]["out"]  # [NCH, P, CHQ, C] f16
        rows = o.transpose(0, 2, 1, 3).reshape(-1, C)[:npx].astype(np.float32)
        unsorted = np.empty_like(rows)
        unsorted[perms[c]] = rows
        r = unsorted.reshape(nb, CROP, CROP, C)
        full[g] = r.transpose(0, 3, 1, 2)
    return full
